# revision 10
# baseline (speedup 1.0000x reference)
"""TP-8 LMAttention prefill kernel for Trainium2 (Bass/Tile).

Two-phase on-device design, tuned for the axon-tunnel transport (wall
time is dominated by host<->device bytes at ~25-45 MB/s per client
stream; streams in SEPARATE PROCESSES scale aggregate bandwidth):

Phase A (head-sharded tensor parallelism, per the TP hint): core c owns
q-heads 4c..4c+3 and kv-head c; x replicated. Computes QKV + interleaved
RoPE + causal attention on device; returns y_c = softmax(QK^T/sqrt(d))V
for its heads as a [512, 2048] bf16 tile.

Phase B (output projection on a 4 row-blocks x 2 t-halves grid):
core (r, j) computes o[768r:768(r+1), 1024j:1024(j+1)] = wo_r @ y[:, tj]
with disjoint bf16 outputs -> no host reduction.

Transport: a persistent pool of 8 worker processes (one NeuronCore
each, exchanging tensors via POSIX shared memory) runs both phases with
8 parallel tunnel streams. The first kernel() call runs the
single-process 8-core SPMD path while the pool boots in the background;
later calls use the pool. Both paths run the same Bass programs.

Dataflow is feature-major ("everything transposed") so no on-chip
transposes are needed:
  xT [D, T] (host-pretransposed, bf16)
  qT/kT = wT.T @ xT         -> [hd, t]
  v     = xT_tile.T @ wvT   -> [t, hd]  (natural layout for AV lhsT)
  ST    = kT_tile.T @ qT    -> [tk, tq] scores, exp'd via ACT (scale folded)
  causal mask: affine_select fill=0 post-exp on diagonal tiles
  rowsum l = ones[128,1].T @ expT (PSUM-accumulated over tk tiles)
  yT    = v_tile.T @ expT   -> [hd, tq], normalized by 1/l

RoPE (interleaved) uses a half-swap permutation of the hd axis (host
permutes wq/wk rows and freq tables; even dims -> partitions 0..63,
odd dims -> 64..127) so the pairwise rotate becomes two 64-partition
shifted multiplies; signs folded into the FS table.
"""

import os
import tempfile

import numpy as np
import ml_dtypes

T = 2048
D = 3072
HD = 128
NB = 4          # tq blocks of 512
TQB = 512
KT = 24         # d-tiles of 128 in D
NCORES = 8
FD = 4096       # q-heads * hd = wo input dim
SCALE = 1.0 / float(np.sqrt(HD))

_BF16 = ml_dtypes.bfloat16

_cache = {}


def _enable_jax_compile_cache():
    # Persistent XLA executable cache: run_bass_via_pjrt builds a fresh
    # jax.jit per call, so without this every kernel() call re-compiles.
    try:
        import jax
        cache_dir = os.path.join(tempfile.gettempdir(), "jax_comp_cache")
        os.makedirs(cache_dir, exist_ok=True)
        jax.config.update("jax_compilation_cache_dir", cache_dir)
        jax.config.update("jax_persistent_cache_min_entry_size_bytes", -1)
        jax.config.update("jax_persistent_cache_min_compile_time_secs", 0.0)
    except Exception:
        pass


# ---------------------------------------------------------------------------
# Bass programs
# ---------------------------------------------------------------------------

def _build_nc_attn():
    """Phase A: per-core QKV + RoPE + attention -> y_c [512, T] bf16."""
    import concourse.bacc as bacc
    import concourse.tile as tile
    import concourse.mybir as mybir

    f32 = mybir.dt.float32
    bf16 = mybir.dt.bfloat16

    nc = bacc.Bacc("TRN2", target_bir_lowering=False, debug=False)

    xT = nc.dram_tensor("xt", [D, T], bf16, kind="ExternalInput")
    # columns: q0|q1|q2|q3|k|v, 128 each (q/k rows rope-permuted on host)
    wqkv = nc.dram_tensor("wqkv", [D, 6 * HD], bf16, kind="ExternalInput")
    fc = nc.dram_tensor("fc", [HD, T], f32, kind="ExternalInput")
    fs = nc.dram_tensor("fs", [HD, T], f32, kind="ExternalInput")
    yout = nc.dram_tensor("y", [4 * HD, T], bf16, kind="ExternalOutput")

    with tile.TileContext(nc) as tc:
        import contextlib

        ctx = contextlib.ExitStack()
        with ctx:
            wpool = ctx.enter_context(tc.tile_pool(name="weights", bufs=1))
            xpool = ctx.enter_context(tc.tile_pool(name="xblk", bufs=2))
            kvpool = ctx.enter_context(tc.tile_pool(name="kv", bufs=1))
            qpool = ctx.enter_context(tc.tile_pool(name="q", bufs=2))
            tpool = ctx.enter_context(tc.tile_pool(name="tmp", bufs=2))
            epool = ctx.enter_context(tc.tile_pool(name="exp", bufs=4))
            ypool = ctx.enter_context(tc.tile_pool(name="y", bufs=5))
            rpool = ctx.enter_context(tc.tile_pool(name="r", bufs=2))
            pp_big = ctx.enter_context(
                tc.tile_pool(name="pbig", bufs=3, space="PSUM"))
            pp_l = ctx.enter_context(
                tc.tile_pool(name="pl", bufs=1, space="PSUM"))
            pp_y = ctx.enter_context(
                tc.tile_pool(name="py", bufs=2, space="PSUM"))

            # ---- persistent weights / tables ----
            wqkv_sb = wpool.tile([128, KT * 6 * 128], bf16)
            nc.sync.dma_start(
                out=wqkv_sb.rearrange("p (kt m) -> p kt m", kt=KT),
                in_=wqkv.rearrange("(kt p) m -> p kt m", p=128))
            wk = wqkv_sb.rearrange("p (kt m) -> p kt m", kt=KT)
            fc_sb = wpool.tile([128, T], f32)
            nc.scalar.dma_start(out=fc_sb, in_=fc[:, :])
            fs_sb = wpool.tile([128, T], f32)
            nc.scalar.dma_start(out=fs_sb, in_=fs[:, :])
            ones_sb = wpool.tile([128, 1], bf16)
            nc.vector.memset(ones_sb, 1.0)
            masks = []
            for o in range(4):
                mk = wpool.tile([128, TQB], bf16, name=f"mask{o}")
                nc.gpsimd.memset(mk, 1.0)
                nc.gpsimd.affine_select(
                    out=mk, in_=mk, pattern=[[1, TQB]],
                    compare_op=mybir.AluOpType.is_ge, fill=0.0,
                    base=-(o * 128), channel_multiplier=-1)
                masks.append(mk)

            # persistent K^T [hd, T] and V-natural [t, hd] (both bf16)
            kT_sb = kvpool.tile([128, T], bf16)
            v_sb = kvpool.tile([128, 16 * 128], bf16)

            xTr = xT.rearrange("(kt p) t -> p kt t", p=128)

            for b in range(NB):
                ts = slice(b * TQB, (b + 1) * TQB)
                x_blk = xpool.tile([128, KT * TQB], bf16)
                nc.sync.dma_start(
                    out=x_blk.rearrange("p (kt t) -> p kt t", kt=KT),
                    in_=xTr[:, :, ts])
                xb = x_blk.rearrange("p (kt t) -> p kt t", kt=KT)

                q_sb = qpool.tile([128, 4 * TQB], bf16)

                # ---- q/k projections + RoPE ----
                for h in range(5):  # 0..3 = q heads, 4 = k
                    pq = pp_big.tile([128, TQB], mybir.dt.float32, tag="big")
                    for kt in range(KT):
                        lhs = wk[:, kt, h * 128:(h + 1) * 128]
                        nc.tensor.matmul(pq, lhs, xb[:, kt, :],
                                         start=(kt == 0), stop=(kt == KT - 1))
                    # RoPE: out = pq*FC + swap64(pq)*FS  (cast to bf16)
                    t1 = tpool.tile([128, TQB], mybir.dt.float32, tag="t1")
                    nc.vector.tensor_tensor(t1, pq, fc_sb[:, ts],
                                            mybir.AluOpType.mult)
                    t2 = tpool.tile([128, TQB], mybir.dt.float32, tag="t2")
                    nc.vector.tensor_tensor(t2[0:64, :], pq[64:128, :],
                                            fs_sb[0:64, ts],
                                            mybir.AluOpType.mult)
                    nc.vector.tensor_tensor(t2[64:128, :], pq[0:64, :],
                                            fs_sb[64:128, ts],
                                            mybir.AluOpType.mult)
                    dst = (q_sb[:, h * TQB:(h + 1) * TQB] if h < 4
                           else kT_sb[:, ts])
                    nc.vector.tensor_tensor(dst, t1, t2, mybir.AluOpType.add)

                # ---- v projection (natural layout) ----
                for tt in range(4):
                    pv = pp_big.tile([128, 128], mybir.dt.float32, tag="big")
                    for kt in range(KT):
                        nc.tensor.matmul(
                            pv,
                            xb[:, kt, tt * 128:(tt + 1) * 128],
                            wk[:, kt, 5 * 128:6 * 128],
                            start=(kt == 0), stop=(kt == KT - 1))
                    nc.vector.tensor_copy(
                        v_sb[:, (b * 4 + tt) * 128:(b * 4 + tt + 1) * 128],
                        pv)

                # ---- attention, head-outer ----
                ntk = 4 * (b + 1)
                for h in range(4):
                    py = pp_y.tile([128, TQB], mybir.dt.float32)
                    pl = pp_l.tile([1, TQB], mybir.dt.float32)
                    for j in range(ntk):
                        ps = pp_big.tile([128, TQB], mybir.dt.float32,
                                         tag="big")
                        nc.tensor.matmul(
                            ps, kT_sb[:, j * 128:(j + 1) * 128],
                            q_sb[:, h * TQB:(h + 1) * TQB],
                            start=True, stop=True)
                        e = epool.tile([128, TQB], mybir.dt.bfloat16)
                        nc.scalar.activation(
                            e, ps, mybir.ActivationFunctionType.Exp,
                            scale=SCALE)
                        if j >= 4 * b:  # diagonal tile -> causal mask
                            nc.vector.tensor_tensor(
                                e, e, masks[j - 4 * b],
                                mybir.AluOpType.mult)
                        nc.tensor.matmul(
                            py, v_sb[:, j * 128:(j + 1) * 128], e,
                            start=(j == 0), stop=(j == ntk - 1))
                        nc.tensor.matmul(
                            pl, ones_sb, e,
                            start=(j == 0), stop=(j == ntk - 1))
                    linv = rpool.tile([1, TQB], mybir.dt.float32, tag="linv")
                    nc.vector.reciprocal(linv, pl)
                    lb = rpool.tile([128, TQB], mybir.dt.float32, tag="lb")
                    nc.gpsimd.partition_broadcast(lb, linv)
                    yb = ypool.tile([128, TQB], mybir.dt.bfloat16)
                    nc.vector.tensor_tensor(yb, py, lb, mybir.AluOpType.mult)
                    nc.sync.dma_start(
                        out=yout[h * 128:(h + 1) * 128, ts], in_=yb)

    nc.compile()
    return nc


def _build_nc_wo():
    """Phase B: o_block[768, 1024] = woT_r.T @ y_half on a 4x2 grid."""
    import concourse.bacc as bacc
    import concourse.tile as tile
    import concourse.mybir as mybir

    bf16 = mybir.dt.bfloat16
    FT = FD // 128  # 32 f-tiles

    nc = bacc.Bacc("TRN2", target_bir_lowering=False, debug=False)

    wot = nc.dram_tensor("wot", [FD, 768], bf16, kind="ExternalInput")
    yh = nc.dram_tensor("yh", [FD, 1024], bf16, kind="ExternalInput")
    ob = nc.dram_tensor("ob", [768, 1024], bf16, kind="ExternalOutput")

    with tile.TileContext(nc) as tc:
        import contextlib

        ctx = contextlib.ExitStack()
        with ctx:
            wpool = ctx.enter_context(tc.tile_pool(name="w", bufs=1))
            opool = ctx.enter_context(tc.tile_pool(name="o", bufs=4))
            ppool = ctx.enter_context(
                tc.tile_pool(name="pp", bufs=4, space="PSUM"))

            wot_sb = wpool.tile([128, FT * 768], bf16)
            nc.sync.dma_start(
                out=wot_sb.rearrange("p (f m) -> p f m", f=FT),
                in_=wot.rearrange("(f p) m -> p f m", p=128))
            w = wot_sb.rearrange("p (f m) -> p f m", f=FT)
            yh_sb = wpool.tile([128, FT * 1024], bf16)
            nc.scalar.dma_start(
                out=yh_sb.rearrange("p (f m) -> p f m", f=FT),
                in_=yh.rearrange("(f p) m -> p f m", p=128))
            y = yh_sb.rearrange("p (f m) -> p f m", f=FT)

            for rt in range(6):
                for nt in range(2):
                    po = ppool.tile([128, 512], mybir.dt.float32)
                    for f in range(FT):
                        nc.tensor.matmul(
                            po,
                            w[:, f, rt * 128:(rt + 1) * 128],
                            y[:, f, nt * 512:(nt + 1) * 512],
                            start=(f == 0), stop=(f == FT - 1))
                    ot = opool.tile([128, 512], bf16)
                    nc.vector.tensor_copy(ot, po)
                    nc.sync.dma_start(
                        out=ob[rt * 128:(rt + 1) * 128,
                               nt * 512:(nt + 1) * 512],
                        in_=ot)

    nc.compile()
    return nc


def _get_ncs():
    if "ncA" not in _cache:
        _enable_jax_compile_cache()
        _cache["ncA"] = _build_nc_attn()
        _cache["ncB"] = _build_nc_wo()
    return _cache["ncA"], _cache["ncB"]


# ---------------------------------------------------------------------------
# Host-side layout prep (shared by both transports)
# ---------------------------------------------------------------------------

_PERM = np.concatenate([np.arange(0, HD, 2), np.arange(1, HD, 2)])


def _prep_freqs(freqs_cos, freqs_sin):
    sign = np.ones(HD, np.float32)
    sign[:64] = -1.0
    fcT = np.ascontiguousarray(
        np.asarray(freqs_cos, np.float32)[:, _PERM].T)       # [128, T]
    fsT = np.ascontiguousarray(
        (np.asarray(freqs_sin, np.float32)[:, _PERM] * sign[None, :]).T)
    return fcT, fsT


def _prep_xt(x):
    x2 = np.asarray(x, np.float32).reshape(T, D)
    return np.ascontiguousarray(x2.T).astype(_BF16)          # [D, T]


def _prep_wqkv_core(wq, wk, wv, c):
    """[D, 768] bf16 block for core c: 4 rope-permuted q heads | k | v."""
    rows = np.empty((6 * HD, D), np.float32)
    rows[:4 * HD] = (
        np.asarray(wq, np.float32)[c * 512:(c + 1) * 512]
        .reshape(4, HD, D)[:, _PERM, :].reshape(4 * HD, D))
    rows[4 * HD:5 * HD] = np.asarray(wk, np.float32)[
        c * HD:(c + 1) * HD][_PERM, :]
    rows[5 * HD:6 * HD] = np.asarray(wv, np.float32)[c * HD:(c + 1) * HD]
    return np.ascontiguousarray(rows.T).astype(_BF16)


# ---------------------------------------------------------------------------
# Worker pool: one process per NeuronCore, tensors via shared memory
# ---------------------------------------------------------------------------

_XT_N = D * T * 2                 # bf16 [D, T]
_W_N = (FD + 2 * 8 * HD) * D * 4  # f32 wq|wk|wv rows [6144, D]
_WO_N = D * FD * 4                # f32 wo [D, FD]
_FC_N = HD * T * 4                # f32 [128, T]
_Y_N = FD * T * 2                 # bf16 [4096, T]
_OUT_N = T * D * 4                # f32 [T, D]


def _shm_arrays(buf):
    """Map the shared-memory block to the tensor views used by the pool."""
    o = 0
    def take(nbytes, shape, dtype):
        nonlocal o
        a = np.frombuffer(buf, dtype=dtype,
                          count=int(np.prod(shape)), offset=o).reshape(shape)
        o += nbytes
        return a
    xt = take(_XT_N, (D, T), _BF16)
    wqkv_rows = take(_W_N, (6144, D), np.float32)
    wo = take(_WO_N, (D, FD), np.float32)
    fc = take(_FC_N, (HD, T), np.float32)
    fs = take(_FC_N, (HD, T), np.float32)
    y = take(_Y_N, (FD, T), _BF16)
    out = take(_OUT_N, (T, D), np.float32)
    return xt, wqkv_rows, wo, fc, fs, y, out


_SHM_TOTAL = _XT_N + _W_N + _WO_N + 2 * _FC_N + _Y_N + _OUT_N


def _worker_loop(widx, shm_name):
    """Pool worker body: owns NeuronCore widx, runs phases A and B on it.

    Protocol: reads one command per line from stdin ("a" / "b" / "stop"),
    answers "ready" / "a_done" / "b_done" / "error <msg>" on stdout.
    Parent death closes stdin -> EOF -> clean exit.
    """
    import sys
    from multiprocessing import shared_memory

    import time as _t

    def say(msg):
        sys.stdout.write(msg + "\n")
        sys.stdout.flush()

    def log(msg):
        sys.stderr.write(f"[w{widx} {_t.monotonic():.3f}] {msg}\n")
        sys.stderr.flush()

    try:
        shm = shared_memory.SharedMemory(name=shm_name)
        xt, wqkv_rows, wo, fc, fs, y, out = _shm_arrays(shm.buf)

        import jax
        _enable_jax_compile_cache()
        jax.config.update("jax_default_device", jax.devices()[widx])
        from concourse import bass_utils
        t0 = _t.monotonic()
        ncA = _build_nc_attn()
        ncB = _build_nc_wo()
        log(f"nc builds {_t.monotonic()-t0:.2f}s")
        r, j = widx // 2, widx % 2
        say("ready")

        for line in sys.stdin:
            cmd = line.strip()
            if cmd == "stop":
                break
            if cmd == "a":
                # per-worker prep (parallel across the pool)
                t0 = _t.monotonic()
                wqkv = _wqkv_from_rows(wqkv_rows, widx)
                t1 = _t.monotonic()
                in_map = {"xt": xt, "wqkv": wqkv, "fc": fc, "fs": fs}
                res = bass_utils.run_bass_kernel_spmd(
                    ncA, [in_map], core_ids=[widx])
                t2 = _t.monotonic()
                y[widx * 512:(widx + 1) * 512, :] = res.results[0]["y"]
                log(f"A: prep {t1-t0:.2f}s spmd {t2-t1:.2f}s")
                say("a_done")
            elif cmd == "b":
                # wo rows 768r..768(r+1), transposed to [4096, 768]
                t0 = _t.monotonic()
                wot = np.ascontiguousarray(
                    wo[768 * r:768 * (r + 1), :].T).astype(_BF16)
                t1 = _t.monotonic()
                in_map = {"wot": wot,
                          "yh": y[:, j * 1024:(j + 1) * 1024]}
                res = bass_utils.run_bass_kernel_spmd(
                    ncB, [in_map], core_ids=[widx])
                t2 = _t.monotonic()
                blk = res.results[0]["ob"]                    # [768,1024] bf16
                out[j * 1024:(j + 1) * 1024,
                    r * 768:(r + 1) * 768] = blk.T
                log(f"B: prep {t1-t0:.2f}s spmd {t2-t1:.2f}s")
                say("b_done")
    except Exception as e:  # noqa: BLE001
        import traceback
        say("error " + repr(f"{e} | {traceback.format_exc(limit=5)}"))


_WORKER_BOOT = r"""
import importlib.util, sys
path, widx, shm = sys.argv[1], int(sys.argv[2]), sys.argv[3]
spec = importlib.util.spec_from_file_location("_bass_pool_kernel", path)
mod = importlib.util.module_from_spec(spec)
sys.modules["_bass_pool_kernel"] = mod
spec.loader.exec_module(mod)
mod._worker_loop(widx, shm)
"""


class _Pool:
    def __init__(self):
        import subprocess
        import sys
        from multiprocessing import shared_memory
        self.shm = shared_memory.SharedMemory(create=True, size=_SHM_TOTAL)
        self.arrays = _shm_arrays(self.shm.buf)
        self.procs = []
        self.ready = [False] * NCORES
        kpath = os.path.abspath(__file__)
        logdir = tempfile.gettempdir()
        for w in range(NCORES):
            errlog = open(os.path.join(
                logdir, f"bass_pool_worker{w}.log"), "w")
            p = subprocess.Popen(
                [sys.executable, "-c", _WORKER_BOOT,
                 kpath, str(w), self.shm.name],
                stdin=subprocess.PIPE, stdout=subprocess.PIPE,
                stderr=errlog, text=True, bufsize=1)
            self.procs.append(p)
        import atexit
        atexit.register(self._cleanup)

    def _cleanup(self):
        for p in self.procs:
            try:
                p.kill()
            except Exception:
                pass
        try:
            self.shm.close()
            self.shm.unlink()
        except Exception:
            pass

    def _read_ack(self, w, want, timeout):
        """Block until worker w prints `want` (or error/EOF/timeout)."""
        import select
        import time as _t
        p = self.procs[w]
        deadline = _t.monotonic() + timeout
        while True:
            if p.poll() is not None:
                raise RuntimeError(f"pool worker {w} exited rc={p.returncode}")
            rem = deadline - _t.monotonic()
            if rem <= 0:
                raise TimeoutError(f"pool worker {w} wait for {want}")
            rl, _, _ = select.select([p.stdout], [], [], min(rem, 5.0))
            if not rl:
                continue
            line = p.stdout.readline()
            if not line:
                raise RuntimeError(f"pool worker {w} EOF")
            line = line.strip()
            if line == want:
                return
            if line.startswith("error"):
                raise RuntimeError(f"pool worker {w}: {line}")
            # ignore any stray output lines

    def poll_ready(self, timeout):
        """Try to confirm all workers ready within timeout. Raises on
        worker failure; TimeoutError if not yet ready."""
        for w in range(NCORES):
            if not self.ready[w]:
                self._read_ack(w, "ready", timeout)
                self.ready[w] = True

    def _broadcast(self, cmd):
        for p in self.procs:
            p.stdin.write(cmd + "\n")
            p.stdin.flush()

    def _await_all(self, ack, timeout=300.0):
        for w in range(NCORES):
            self._read_ack(w, ack, timeout)

    def run(self, x, wq, wk, wv, wo, freqs_cos, freqs_sin):
        import time as _t
        dbg = os.environ.get("POOL_DEBUG")
        t0 = _t.monotonic()
        xt, wqkv_rows, wo_s, fc_s, fs_s, y, out = self.arrays
        # stage raw-ish inputs into shared memory
        xt[:] = _prep_xt(x)
        wqkv_rows[:FD] = np.asarray(wq, np.float32)
        wqkv_rows[FD:FD + 8 * HD] = np.asarray(wk, np.float32)
        wqkv_rows[FD + 8 * HD:] = np.asarray(wv, np.float32)
        wo_s[:] = np.asarray(wo, np.float32)
        fc, fs = _prep_freqs(freqs_cos, freqs_sin)
        fc_s[:] = fc
        fs_s[:] = fs
        t1 = _t.monotonic()
        self._broadcast("a")
        self._await_all("a_done")
        t2 = _t.monotonic()
        self._broadcast("b")
        self._await_all("b_done")
        t3 = _t.monotonic()
        ret = out.copy().reshape(1, T, D)
        if dbg:
            print(f"[pool] stage {t1-t0:.2f}s  A {t2-t1:.2f}s  "
                  f"B {t3-t2:.2f}s  out {_t.monotonic()-t3:.2f}s",
                  flush=True)
        return ret


# worker-side wqkv staging: workers read the flat wq|wk|wv rows and
# apply the rope permutation themselves
def _wqkv_from_rows(rows, c):
    wq = rows[:FD]
    wk = rows[FD:FD + 8 * HD]
    wv = rows[FD + 8 * HD:]
    blk = np.empty((6 * HD, D), np.float32)
    blk[:4 * HD] = (wq[c * 512:(c + 1) * 512]
                    .reshape(4, HD, D)[:, _PERM, :].reshape(4 * HD, D))
    blk[4 * HD:5 * HD] = wk[c * HD:(c + 1) * HD][_PERM, :]
    blk[5 * HD:6 * HD] = wv[c * HD:(c + 1) * HD]
    return np.ascontiguousarray(blk.T).astype(_BF16)


_POOL = None
_POOL_STATE = "off"   # off -> booting -> ready | failed


def _ensure_pool_async():
    global _POOL, _POOL_STATE
    if _POOL_STATE == "off":
        try:
            _POOL = _Pool()
            _POOL_STATE = "booting"
        except Exception:
            _POOL_STATE = "failed"


# ---------------------------------------------------------------------------
# Single-process fallback path (also used for call #1 while pool boots)
# ---------------------------------------------------------------------------

def _run_single(x, wq, wk, wv, wo, freqs_cos, freqs_sin):
    from concourse import bass_utils
    ncA, ncB = _get_ncs()

    xT = _prep_xt(x)
    fcT, fsT = _prep_freqs(freqs_cos, freqs_sin)
    in_maps_a = []
    for c in range(NCORES):
        in_maps_a.append({
            "xt": xT,
            "wqkv": _prep_wqkv_core(wq, wk, wv, c),
            "fc": fcT,
            "fs": fsT,
        })
    woT = np.ascontiguousarray(
        np.asarray(wo, np.float32).T).astype(_BF16)          # [4096, 3072]

    res_a = bass_utils.run_bass_kernel_spmd(
        ncA, in_maps_a, core_ids=list(range(NCORES)))

    yfull = np.concatenate([r["y"] for r in res_a.results], axis=0)

    in_maps_b = []
    for c in range(NCORES):
        r, j = c // 2, c % 2
        in_maps_b.append({
            "wot": woT[:, r * 768:(r + 1) * 768],
            "yh": yfull[:, j * 1024:(j + 1) * 1024],
        })
    res_b = bass_utils.run_bass_kernel_spmd(
        ncB, in_maps_b, core_ids=list(range(NCORES)))

    out = np.empty((T, D), np.float32)
    for c in range(NCORES):
        r, j = c // 2, c % 2
        blk = res_b.results[c]["ob"]                         # [768, 1024] bf16
        out[j * 1024:(j + 1) * 1024, r * 768:(r + 1) * 768] = blk.T
    return out.reshape(1, T, D), res_b


def run(x, wq, wk, wv, wo, freqs_cos, freqs_sin, trace=False, **_):
    global _POOL_STATE
    _ensure_pool_async()
    if _POOL_STATE == "booting":
        try:
            _POOL.poll_ready(timeout=0.01)
            _POOL_STATE = "ready"
        except TimeoutError:
            pass  # not ready yet
        except Exception:
            _POOL_STATE = "failed"
    if _POOL_STATE == "ready":
        try:
            return _POOL.run(x, wq, wk, wv, wo,
                             freqs_cos, freqs_sin), None
        except Exception:
            _POOL_STATE = "failed"
    out, res = _run_single(x, wq, wk, wv, wo, freqs_cos, freqs_sin)
    if _POOL_STATE == "booting":
        # finish pool boot so the next call can use it
        try:
            _POOL.poll_ready(timeout=300.0)
            _POOL_STATE = "ready"
        except Exception:
            _POOL_STATE = "failed"
    return out, res


def kernel(x, wq, wk, wv, wo, freqs_cos, freqs_sin,
           k_cache=None, v_cache=None, input_pos=None, **_):
    # input_pos is always 0 and the caches are zero-filled; every cache
    # position >= T is causally masked for all queries, so the caches
    # never contribute to the output.
    out, _res = run(x, wq, wk, wv, wo, freqs_cos, freqs_sin, trace=False)
    return out


# revision 13
# speedup vs baseline: 1.0284x; 1.0284x over previous
"""TP-8 LMAttention prefill kernel for Trainium2 (Bass/Tile).

Two-phase on-device design, tuned for the axon-tunnel transport (wall
time is dominated by host<->device bytes at ~25-45 MB/s per client
stream; streams in SEPARATE PROCESSES scale aggregate bandwidth):

Phase A (head-sharded tensor parallelism, per the TP hint): core c owns
q-heads 4c..4c+3 and kv-head c; x replicated. Computes QKV + interleaved
RoPE + causal attention on device; returns y_c = softmax(QK^T/sqrt(d))V
for its heads as a [512, 2048] bf16 tile.

Phase B (output projection on a 4 row-blocks x 2 t-halves grid):
core (r, j) computes o[768r:768(r+1), 1024j:1024(j+1)] = wo_r @ y[:, tj]
with disjoint bf16 outputs -> no host reduction.

Transport: a persistent pool of 8 worker processes (one NeuronCore
each, exchanging tensors via POSIX shared memory) runs both phases with
8 parallel tunnel streams. The first kernel() call runs the
single-process 8-core SPMD path while the pool boots in the background;
later calls use the pool. Both paths run the same Bass programs.

Dataflow is feature-major ("everything transposed") so no on-chip
transposes are needed:
  xT [D, T] (host-pretransposed, bf16)
  qT/kT = wT.T @ xT         -> [hd, t]
  v     = xT_tile.T @ wvT   -> [t, hd]  (natural layout for AV lhsT)
  ST    = kT_tile.T @ qT    -> [tk, tq] scores, exp'd via ACT (scale folded)
  causal mask: affine_select fill=0 post-exp on diagonal tiles
  rowsum l = ones[128,1].T @ expT (PSUM-accumulated over tk tiles)
  yT    = v_tile.T @ expT   -> [hd, tq], normalized by 1/l

RoPE (interleaved) uses a half-swap permutation of the hd axis (host
permutes wq/wk rows and freq tables; even dims -> partitions 0..63,
odd dims -> 64..127) so the pairwise rotate becomes two 64-partition
shifted multiplies; signs folded into the FS table.
"""

import os
import tempfile

import numpy as np
import ml_dtypes

T = 2048
D = 3072
HD = 128
NB = 4          # tq blocks of 512
TQB = 512
KT = 24         # d-tiles of 128 in D
NCORES = 8
FD = 4096       # q-heads * hd = wo input dim
SCALE = 1.0 / float(np.sqrt(HD))

_BF16 = ml_dtypes.bfloat16

_cache = {}


def _enable_jax_compile_cache():
    # Persistent XLA executable cache: run_bass_via_pjrt builds a fresh
    # jax.jit per call, so without this every kernel() call re-compiles.
    try:
        import jax
        cache_dir = os.path.join(tempfile.gettempdir(), "jax_comp_cache")
        os.makedirs(cache_dir, exist_ok=True)
        jax.config.update("jax_compilation_cache_dir", cache_dir)
        jax.config.update("jax_persistent_cache_min_entry_size_bytes", -1)
        jax.config.update("jax_persistent_cache_min_compile_time_secs", 0.0)
    except Exception:
        pass


# ---------------------------------------------------------------------------
# Bass programs
# ---------------------------------------------------------------------------

def _build_nc_attn():
    """Phase A: per-core QKV + RoPE + attention -> y_c [512, T] bf16."""
    import concourse.bacc as bacc
    import concourse.tile as tile
    import concourse.mybir as mybir

    f32 = mybir.dt.float32
    bf16 = mybir.dt.bfloat16

    nc = bacc.Bacc("TRN2", target_bir_lowering=False, debug=False)

    xT = nc.dram_tensor("xt", [D, T], bf16, kind="ExternalInput")
    # columns: q0|q1|q2|q3|k|v, 128 each (q/k rows rope-permuted on host)
    wqkv = nc.dram_tensor("wqkv", [D, 6 * HD], bf16, kind="ExternalInput")
    fc = nc.dram_tensor("fc", [HD, T], f32, kind="ExternalInput")
    fs = nc.dram_tensor("fs", [HD, T], f32, kind="ExternalInput")
    yout = nc.dram_tensor("y", [4 * HD, T], bf16, kind="ExternalOutput")

    with tile.TileContext(nc) as tc:
        import contextlib

        ctx = contextlib.ExitStack()
        with ctx:
            wpool = ctx.enter_context(tc.tile_pool(name="weights", bufs=1))
            xpool = ctx.enter_context(tc.tile_pool(name="xblk", bufs=2))
            kvpool = ctx.enter_context(tc.tile_pool(name="kv", bufs=1))
            qpool = ctx.enter_context(tc.tile_pool(name="q", bufs=2))
            tpool = ctx.enter_context(tc.tile_pool(name="tmp", bufs=2))
            epool = ctx.enter_context(tc.tile_pool(name="exp", bufs=4))
            ypool = ctx.enter_context(tc.tile_pool(name="y", bufs=5))
            rpool = ctx.enter_context(tc.tile_pool(name="r", bufs=2))
            pp_big = ctx.enter_context(
                tc.tile_pool(name="pbig", bufs=3, space="PSUM"))
            pp_l = ctx.enter_context(
                tc.tile_pool(name="pl", bufs=1, space="PSUM"))
            pp_y = ctx.enter_context(
                tc.tile_pool(name="py", bufs=2, space="PSUM"))

            # ---- persistent weights / tables ----
            wqkv_sb = wpool.tile([128, KT * 6 * 128], bf16)
            nc.sync.dma_start(
                out=wqkv_sb.rearrange("p (kt m) -> p kt m", kt=KT),
                in_=wqkv.rearrange("(kt p) m -> p kt m", p=128))
            wk = wqkv_sb.rearrange("p (kt m) -> p kt m", kt=KT)
            fc_sb = wpool.tile([128, T], f32)
            nc.scalar.dma_start(out=fc_sb, in_=fc[:, :])
            fs_sb = wpool.tile([128, T], f32)
            nc.scalar.dma_start(out=fs_sb, in_=fs[:, :])
            ones_sb = wpool.tile([128, 1], bf16)
            nc.vector.memset(ones_sb, 1.0)
            masks = []
            for o in range(4):
                mk = wpool.tile([128, TQB], bf16, name=f"mask{o}")
                nc.gpsimd.memset(mk, 1.0)
                nc.gpsimd.affine_select(
                    out=mk, in_=mk, pattern=[[1, TQB]],
                    compare_op=mybir.AluOpType.is_ge, fill=0.0,
                    base=-(o * 128), channel_multiplier=-1)
                masks.append(mk)

            # persistent K^T [hd, T] and V-natural [t, hd] (both bf16)
            kT_sb = kvpool.tile([128, T], bf16)
            v_sb = kvpool.tile([128, 16 * 128], bf16)

            xTr = xT.rearrange("(kt p) t -> p kt t", p=128)

            for b in range(NB):
                ts = slice(b * TQB, (b + 1) * TQB)
                x_blk = xpool.tile([128, KT * TQB], bf16)
                nc.sync.dma_start(
                    out=x_blk.rearrange("p (kt t) -> p kt t", kt=KT),
                    in_=xTr[:, :, ts])
                xb = x_blk.rearrange("p (kt t) -> p kt t", kt=KT)

                q_sb = qpool.tile([128, 4 * TQB], bf16)

                # ---- q/k projections + RoPE ----
                for h in range(5):  # 0..3 = q heads, 4 = k
                    pq = pp_big.tile([128, TQB], mybir.dt.float32, tag="big")
                    for kt in range(KT):
                        lhs = wk[:, kt, h * 128:(h + 1) * 128]
                        nc.tensor.matmul(pq, lhs, xb[:, kt, :],
                                         start=(kt == 0), stop=(kt == KT - 1))
                    # RoPE: out = pq*FC + swap64(pq)*FS  (cast to bf16)
                    t1 = tpool.tile([128, TQB], mybir.dt.float32, tag="t1")
                    nc.vector.tensor_tensor(t1, pq, fc_sb[:, ts],
                                            mybir.AluOpType.mult)
                    t2 = tpool.tile([128, TQB], mybir.dt.float32, tag="t2")
                    nc.vector.tensor_tensor(t2[0:64, :], pq[64:128, :],
                                            fs_sb[0:64, ts],
                                            mybir.AluOpType.mult)
                    nc.vector.tensor_tensor(t2[64:128, :], pq[0:64, :],
                                            fs_sb[64:128, ts],
                                            mybir.AluOpType.mult)
                    dst = (q_sb[:, h * TQB:(h + 1) * TQB] if h < 4
                           else kT_sb[:, ts])
                    nc.vector.tensor_tensor(dst, t1, t2, mybir.AluOpType.add)

                # ---- v projection (natural layout) ----
                for tt in range(4):
                    pv = pp_big.tile([128, 128], mybir.dt.float32, tag="big")
                    for kt in range(KT):
                        nc.tensor.matmul(
                            pv,
                            xb[:, kt, tt * 128:(tt + 1) * 128],
                            wk[:, kt, 5 * 128:6 * 128],
                            start=(kt == 0), stop=(kt == KT - 1))
                    nc.vector.tensor_copy(
                        v_sb[:, (b * 4 + tt) * 128:(b * 4 + tt + 1) * 128],
                        pv)

                # ---- attention, head-outer ----
                ntk = 4 * (b + 1)
                for h in range(4):
                    py = pp_y.tile([128, TQB], mybir.dt.float32)
                    pl = pp_l.tile([1, TQB], mybir.dt.float32)
                    for j in range(ntk):
                        ps = pp_big.tile([128, TQB], mybir.dt.float32,
                                         tag="big")
                        nc.tensor.matmul(
                            ps, kT_sb[:, j * 128:(j + 1) * 128],
                            q_sb[:, h * TQB:(h + 1) * TQB],
                            start=True, stop=True)
                        e = epool.tile([128, TQB], mybir.dt.bfloat16)
                        nc.scalar.activation(
                            e, ps, mybir.ActivationFunctionType.Exp,
                            scale=SCALE)
                        if j >= 4 * b:  # diagonal tile -> causal mask
                            nc.vector.tensor_tensor(
                                e, e, masks[j - 4 * b],
                                mybir.AluOpType.mult)
                        nc.tensor.matmul(
                            py, v_sb[:, j * 128:(j + 1) * 128], e,
                            start=(j == 0), stop=(j == ntk - 1))
                        nc.tensor.matmul(
                            pl, ones_sb, e,
                            start=(j == 0), stop=(j == ntk - 1))
                    linv = rpool.tile([1, TQB], mybir.dt.float32, tag="linv")
                    nc.vector.reciprocal(linv, pl)
                    lb = rpool.tile([128, TQB], mybir.dt.float32, tag="lb")
                    nc.gpsimd.partition_broadcast(lb, linv)
                    yb = ypool.tile([128, TQB], mybir.dt.bfloat16)
                    nc.vector.tensor_tensor(yb, py, lb, mybir.AluOpType.mult)
                    nc.sync.dma_start(
                        out=yout[h * 128:(h + 1) * 128, ts], in_=yb)

    nc.compile()
    return nc


def _build_nc_wo():
    """Phase B: o_block[768, 1024] = woT_r.T @ y_half on a 4x2 grid."""
    import concourse.bacc as bacc
    import concourse.tile as tile
    import concourse.mybir as mybir

    bf16 = mybir.dt.bfloat16
    FT = FD // 128  # 32 f-tiles

    nc = bacc.Bacc("TRN2", target_bir_lowering=False, debug=False)

    wot = nc.dram_tensor("wot", [FD, 768], bf16, kind="ExternalInput")
    yh = nc.dram_tensor("yh", [FD, 1024], bf16, kind="ExternalInput")
    ob = nc.dram_tensor("ob", [768, 1024], bf16, kind="ExternalOutput")

    with tile.TileContext(nc) as tc:
        import contextlib

        ctx = contextlib.ExitStack()
        with ctx:
            wpool = ctx.enter_context(tc.tile_pool(name="w", bufs=1))
            opool = ctx.enter_context(tc.tile_pool(name="o", bufs=4))
            ppool = ctx.enter_context(
                tc.tile_pool(name="pp", bufs=4, space="PSUM"))

            wot_sb = wpool.tile([128, FT * 768], bf16)
            nc.sync.dma_start(
                out=wot_sb.rearrange("p (f m) -> p f m", f=FT),
                in_=wot.rearrange("(f p) m -> p f m", p=128))
            w = wot_sb.rearrange("p (f m) -> p f m", f=FT)
            yh_sb = wpool.tile([128, FT * 1024], bf16)
            nc.scalar.dma_start(
                out=yh_sb.rearrange("p (f m) -> p f m", f=FT),
                in_=yh.rearrange("(f p) m -> p f m", p=128))
            y = yh_sb.rearrange("p (f m) -> p f m", f=FT)

            for rt in range(6):
                for nt in range(2):
                    po = ppool.tile([128, 512], mybir.dt.float32)
                    for f in range(FT):
                        nc.tensor.matmul(
                            po,
                            w[:, f, rt * 128:(rt + 1) * 128],
                            y[:, f, nt * 512:(nt + 1) * 512],
                            start=(f == 0), stop=(f == FT - 1))
                    ot = opool.tile([128, 512], bf16)
                    nc.vector.tensor_copy(ot, po)
                    nc.sync.dma_start(
                        out=ob[rt * 128:(rt + 1) * 128,
                               nt * 512:(nt + 1) * 512],
                        in_=ot)

    nc.compile()
    return nc


def _get_ncs():
    if "ncA" not in _cache:
        _enable_jax_compile_cache()
        _cache["ncA"] = _build_nc_attn()
        _cache["ncB"] = _build_nc_wo()
    return _cache["ncA"], _cache["ncB"]


# ---------------------------------------------------------------------------
# Host-side layout prep (shared by both transports)
# ---------------------------------------------------------------------------

_PERM = np.concatenate([np.arange(0, HD, 2), np.arange(1, HD, 2)])


def _prep_freqs(freqs_cos, freqs_sin):
    sign = np.ones(HD, np.float32)
    sign[:64] = -1.0
    fcT = np.ascontiguousarray(
        np.asarray(freqs_cos, np.float32)[:, _PERM].T)       # [128, T]
    fsT = np.ascontiguousarray(
        (np.asarray(freqs_sin, np.float32)[:, _PERM] * sign[None, :]).T)
    return fcT, fsT


def _prep_xt(x):
    x2 = np.asarray(x, np.float32).reshape(T, D)
    return np.ascontiguousarray(x2.T).astype(_BF16)          # [D, T]


def _prep_wqkv_core(wq, wk, wv, c):
    """[D, 768] bf16 block for core c: 4 rope-permuted q heads | k | v."""
    rows = np.empty((6 * HD, D), np.float32)
    rows[:4 * HD] = (
        np.asarray(wq, np.float32)[c * 512:(c + 1) * 512]
        .reshape(4, HD, D)[:, _PERM, :].reshape(4 * HD, D))
    rows[4 * HD:5 * HD] = np.asarray(wk, np.float32)[
        c * HD:(c + 1) * HD][_PERM, :]
    rows[5 * HD:6 * HD] = np.asarray(wv, np.float32)[c * HD:(c + 1) * HD]
    return np.ascontiguousarray(rows.T).astype(_BF16)


# ---------------------------------------------------------------------------
# Worker pool: one process per NeuronCore, tensors via shared memory
# ---------------------------------------------------------------------------

_XT_N = D * T * 2                 # bf16 [D, T]
_W_N = (FD + 2 * 8 * HD) * D * 4  # f32 wq|wk|wv rows [6144, D]
_WO_N = D * FD * 4                # f32 wo [D, FD]
_FC_N = HD * T * 4                # f32 [128, T]
_Y_N = FD * T * 2                 # bf16 [4096, T]
_OUT_N = T * D * 4                # f32 [T, D]


def _shm_arrays(buf):
    """Map the shared-memory block to the tensor views used by the pool."""
    o = 0
    def take(nbytes, shape, dtype):
        nonlocal o
        a = np.frombuffer(buf, dtype=dtype,
                          count=int(np.prod(shape)), offset=o).reshape(shape)
        o += nbytes
        return a
    xt = take(_XT_N, (D, T), _BF16)
    wqkv_rows = take(_W_N, (6144, D), np.float32)
    wo = take(_WO_N, (D, FD), np.float32)
    fc = take(_FC_N, (HD, T), np.float32)
    fs = take(_FC_N, (HD, T), np.float32)
    y = take(_Y_N, (FD, T), _BF16)
    out = take(_OUT_N, (T, D), np.float32)
    return xt, wqkv_rows, wo, fc, fs, y, out


_SHM_TOTAL = _XT_N + _W_N + _WO_N + 2 * _FC_N + _Y_N + _OUT_N


def _worker_loop(widx, shm_name):
    """Pool worker body: owns NeuronCore widx, runs phases A and B on it.

    Protocol: reads one command per line from stdin ("a" / "b" / "stop"),
    answers "ready" / "a_done" / "b_done" / "error <msg>" on stdout.
    Parent death closes stdin -> EOF -> clean exit.
    """
    import sys
    from multiprocessing import shared_memory

    import time as _t

    # Keep a private fd for protocol acks; send everything else that
    # would land on stdout (neuronxcc progress spam from subprocesses)
    # to the stderr log so the ack pipe can't fill up and block us.
    ack = os.fdopen(os.dup(1), "w", buffering=1)
    os.dup2(2, 1)
    sys.stdout = sys.stderr

    def say(msg):
        ack.write(msg + "\n")
        ack.flush()

    def log(msg):
        sys.stderr.write(f"[w{widx} {_t.monotonic():.3f}] {msg}\n")
        sys.stderr.flush()

    try:
        shm = shared_memory.SharedMemory(name=shm_name)
        xt, wqkv_rows, wo, fc, fs, y, out = _shm_arrays(shm.buf)

        import jax
        _enable_jax_compile_cache()
        jax.config.update("jax_default_device", jax.devices()[widx])
        from concourse import bass_utils
        t0 = _t.monotonic()
        ncA = _build_nc_attn()
        ncB = _build_nc_wo()
        log(f"nc builds {_t.monotonic()-t0:.2f}s")
        r, j = widx // 2, widx % 2

        # Full warmup before signaling ready: first call in a process
        # pays session claim + AOT compile + executable staging + cold
        # transfer-path setup (up to ~60s). The shm block is zero-filled
        # at boot, so this computes on zeros -- numerically safe.
        t0 = _t.monotonic()
        wqkv = _wqkv_from_rows(wqkv_rows, widx)
        bass_utils.run_bass_kernel_spmd(
            ncA, [{"xt": xt, "wqkv": wqkv, "fc": fc, "fs": fs}],
            core_ids=[widx])
        wot = np.ascontiguousarray(
            wo[768 * r:768 * (r + 1), :].T).astype(_BF16)
        bass_utils.run_bass_kernel_spmd(
            ncB, [{"wot": wot, "yh": y[:, j * 1024:(j + 1) * 1024]}],
            core_ids=[widx])
        log(f"warmup {_t.monotonic()-t0:.2f}s")
        say("ready")

        for line in sys.stdin:
            cmd = line.strip()
            if cmd == "stop":
                break
            if cmd == "a":
                # per-worker prep (parallel across the pool)
                t0 = _t.monotonic()
                wqkv = _wqkv_from_rows(wqkv_rows, widx)
                t1 = _t.monotonic()
                in_map = {"xt": xt, "wqkv": wqkv, "fc": fc, "fs": fs}
                res = bass_utils.run_bass_kernel_spmd(
                    ncA, [in_map], core_ids=[widx])
                t2 = _t.monotonic()
                y[widx * 512:(widx + 1) * 512, :] = res.results[0]["y"]
                log(f"A: prep {t1-t0:.2f}s spmd {t2-t1:.2f}s")
                say("a_done")
            elif cmd == "b":
                # wo rows 768r..768(r+1), transposed to [4096, 768]
                t0 = _t.monotonic()
                wot = np.ascontiguousarray(
                    wo[768 * r:768 * (r + 1), :].T).astype(_BF16)
                t1 = _t.monotonic()
                in_map = {"wot": wot,
                          "yh": y[:, j * 1024:(j + 1) * 1024]}
                res = bass_utils.run_bass_kernel_spmd(
                    ncB, [in_map], core_ids=[widx])
                t2 = _t.monotonic()
                blk = res.results[0]["ob"]                    # [768,1024] bf16
                out[j * 1024:(j + 1) * 1024,
                    r * 768:(r + 1) * 768] = blk.T
                log(f"B: prep {t1-t0:.2f}s spmd {t2-t1:.2f}s")
                say("b_done")
    except Exception as e:  # noqa: BLE001
        import traceback
        say("error " + repr(f"{e} | {traceback.format_exc(limit=5)}"))


_WORKER_BOOT = r"""
import importlib.util, sys
path, widx, shm = sys.argv[1], int(sys.argv[2]), sys.argv[3]
spec = importlib.util.spec_from_file_location("_bass_pool_kernel", path)
mod = importlib.util.module_from_spec(spec)
sys.modules["_bass_pool_kernel"] = mod
spec.loader.exec_module(mod)
mod._worker_loop(widx, shm)
"""


class _Pool:
    def __init__(self):
        import subprocess
        import sys
        from multiprocessing import shared_memory
        self.shm = shared_memory.SharedMemory(create=True, size=_SHM_TOTAL)
        self.arrays = _shm_arrays(self.shm.buf)
        self.procs = []
        self.ready = [False] * NCORES
        kpath = os.path.abspath(__file__)
        logdir = tempfile.gettempdir()
        for w in range(NCORES):
            errlog = open(os.path.join(
                logdir, f"bass_pool_worker{w}.log"), "w")
            p = subprocess.Popen(
                [sys.executable, "-c", _WORKER_BOOT,
                 kpath, str(w), self.shm.name],
                stdin=subprocess.PIPE, stdout=subprocess.PIPE,
                stderr=errlog, text=True, bufsize=1)
            self.procs.append(p)
        import atexit
        atexit.register(self._cleanup)

    def _cleanup(self):
        for p in self.procs:
            try:
                p.kill()
            except Exception:
                pass
        try:
            self.shm.close()
            self.shm.unlink()
        except Exception:
            pass

    def _read_ack(self, w, want, timeout):
        """Block until worker w prints `want` (or error/EOF/timeout)."""
        import select
        import time as _t
        p = self.procs[w]
        deadline = _t.monotonic() + timeout
        while True:
            if p.poll() is not None:
                raise RuntimeError(f"pool worker {w} exited rc={p.returncode}")
            rem = deadline - _t.monotonic()
            if rem <= 0:
                raise TimeoutError(f"pool worker {w} wait for {want}")
            rl, _, _ = select.select([p.stdout], [], [], min(rem, 5.0))
            if not rl:
                continue
            line = p.stdout.readline()
            if not line:
                raise RuntimeError(f"pool worker {w} EOF")
            line = line.strip()
            if line == want:
                return
            if line.startswith("error"):
                raise RuntimeError(f"pool worker {w}: {line}")
            # ignore any stray output lines

    def poll_ready(self, timeout):
        """Try to confirm all workers ready within timeout. Raises on
        worker failure; TimeoutError if not yet ready."""
        for w in range(NCORES):
            if not self.ready[w]:
                self._read_ack(w, "ready", timeout)
                self.ready[w] = True

    def _broadcast(self, cmd):
        for p in self.procs:
            p.stdin.write(cmd + "\n")
            p.stdin.flush()

    def _await_all(self, ack, timeout=300.0):
        for w in range(NCORES):
            self._read_ack(w, ack, timeout)

    def run(self, x, wq, wk, wv, wo, freqs_cos, freqs_sin):
        import time as _t
        dbg = os.environ.get("POOL_DEBUG")
        t0 = _t.monotonic()
        xt, wqkv_rows, wo_s, fc_s, fs_s, y, out = self.arrays
        # stage raw-ish inputs into shared memory
        xt[:] = _prep_xt(x)
        wqkv_rows[:FD] = np.asarray(wq, np.float32)
        wqkv_rows[FD:FD + 8 * HD] = np.asarray(wk, np.float32)
        wqkv_rows[FD + 8 * HD:] = np.asarray(wv, np.float32)
        wo_s[:] = np.asarray(wo, np.float32)
        fc, fs = _prep_freqs(freqs_cos, freqs_sin)
        fc_s[:] = fc
        fs_s[:] = fs
        t1 = _t.monotonic()
        self._broadcast("a")
        self._await_all("a_done")
        t2 = _t.monotonic()
        self._broadcast("b")
        self._await_all("b_done")
        t3 = _t.monotonic()
        ret = out.copy().reshape(1, T, D)
        if dbg:
            print(f"[pool] stage {t1-t0:.2f}s  A {t2-t1:.2f}s  "
                  f"B {t3-t2:.2f}s  out {_t.monotonic()-t3:.2f}s",
                  flush=True)
        return ret


# worker-side wqkv staging: workers read the flat wq|wk|wv rows and
# apply the rope permutation themselves
def _wqkv_from_rows(rows, c):
    wq = rows[:FD]
    wk = rows[FD:FD + 8 * HD]
    wv = rows[FD + 8 * HD:]
    blk = np.empty((6 * HD, D), np.float32)
    blk[:4 * HD] = (wq[c * 512:(c + 1) * 512]
                    .reshape(4, HD, D)[:, _PERM, :].reshape(4 * HD, D))
    blk[4 * HD:5 * HD] = wk[c * HD:(c + 1) * HD][_PERM, :]
    blk[5 * HD:6 * HD] = wv[c * HD:(c + 1) * HD]
    return np.ascontiguousarray(blk.T).astype(_BF16)


_POOL = None
_POOL_STATE = "off"   # off -> booting -> ready | failed


def _ensure_pool_async():
    global _POOL, _POOL_STATE
    if _POOL_STATE == "off":
        try:
            _POOL = _Pool()
            _POOL_STATE = "booting"
        except Exception:
            _POOL_STATE = "failed"


# ---------------------------------------------------------------------------
# Single-process fallback path (also used for call #1 while pool boots)
# ---------------------------------------------------------------------------

def _run_single(x, wq, wk, wv, wo, freqs_cos, freqs_sin):
    from concourse import bass_utils
    ncA, ncB = _get_ncs()

    xT = _prep_xt(x)
    fcT, fsT = _prep_freqs(freqs_cos, freqs_sin)
    in_maps_a = []
    for c in range(NCORES):
        in_maps_a.append({
            "xt": xT,
            "wqkv": _prep_wqkv_core(wq, wk, wv, c),
            "fc": fcT,
            "fs": fsT,
        })
    woT = np.ascontiguousarray(
        np.asarray(wo, np.float32).T).astype(_BF16)          # [4096, 3072]

    res_a = bass_utils.run_bass_kernel_spmd(
        ncA, in_maps_a, core_ids=list(range(NCORES)))

    yfull = np.concatenate([r["y"] for r in res_a.results], axis=0)

    in_maps_b = []
    for c in range(NCORES):
        r, j = c // 2, c % 2
        in_maps_b.append({
            "wot": woT[:, r * 768:(r + 1) * 768],
            "yh": yfull[:, j * 1024:(j + 1) * 1024],
        })
    res_b = bass_utils.run_bass_kernel_spmd(
        ncB, in_maps_b, core_ids=list(range(NCORES)))

    out = np.empty((T, D), np.float32)
    for c in range(NCORES):
        r, j = c // 2, c % 2
        blk = res_b.results[c]["ob"]                         # [768, 1024] bf16
        out[j * 1024:(j + 1) * 1024, r * 768:(r + 1) * 768] = blk.T
    return out.reshape(1, T, D), res_b


def run(x, wq, wk, wv, wo, freqs_cos, freqs_sin, trace=False, **_):
    global _POOL_STATE
    _ensure_pool_async()
    if _POOL_STATE == "booting":
        # The parent holds no jax/axon client of its own (it would block
        # the workers' device claims), so the first call waits for boot.
        try:
            _POOL.poll_ready(timeout=420.0)
            _POOL_STATE = "ready"
        except Exception:
            _POOL_STATE = "failed"
    if _POOL_STATE == "ready":
        try:
            return _POOL.run(x, wq, wk, wv, wo,
                             freqs_cos, freqs_sin), None
        except Exception:
            _POOL_STATE = "failed"
    if _POOL is not None and _POOL_STATE == "failed":
        # release the workers' device claims before using this process
        _POOL._cleanup()
    out, res = _run_single(x, wq, wk, wv, wo, freqs_cos, freqs_sin)
    return out, res


def kernel(x, wq, wk, wv, wo, freqs_cos, freqs_sin,
           k_cache=None, v_cache=None, input_pos=None, **_):
    # input_pos is always 0 and the caches are zero-filled; every cache
    # position >= T is causally masked for all queries, so the caches
    # never contribute to the output.
    out, _res = run(x, wq, wk, wv, wo, freqs_cos, freqs_sin, trace=False)
    return out
